# revision 31
# baseline (speedup 1.0000x reference)
"""Trainium2 Bass kernel for nn_ConstraintWholePoseScoringModule.

The module scores 3 hardcoded harmonic distance constraints (all on pose 0),
scatter-adds the scores into a [nposes, nblocks, nblocks] block-score matrix,
then sums that matrix per pose -> output [1, nposes].  The scatter + full sum
is algebraically a weighted sum of the constraint scores per pose, so the
kernel never materialises the block-score matrix.

Sharding (per the data-parallel hint): pose dimension split across 8 cores,
2 poses per core, no cross-core communication.  Every core runs the same
program on its shard:

  1. DMA the first 3 block offsets of its local pose 0 (the only pose that
     can host constraints, per the module's constant table).
  2. Two indirect-DMA gathers fetch the constraint endpoint atoms straight
     from DRAM coords (row = block_coord_offset[r] + atom; the +1 atom
     offset of the B endpoints is folded into the DMA element_offset).
  3. diff -> squared-norm (fused square+accumulate) -> sqrt -> (d-4)^2.
  4. A tiny matmul contracts the 2 distance-slot scores against a per-core
     [slot, local_pose] weight table (zeros on cores with no constraints),
     giving that core's [1, 2] per-pose output.

Host side only slices inputs per core and concatenates the [2]-vectors.
"""

import sys

sys.path.insert(0, "/opt/trn_rl_repo")

import numpy as np

NCORES = 8
NPOSES = 16
NBLOCKS = 1024
ATOMS_PER_BLOCK = 16
NATOMS = NBLOCKS * ATOMS_PER_BLOCK  # 16384
PLOC = NPOSES // NCORES  # poses per core = 2
IDEAL = 4.0

# Constant constraint table of the torch module: (pose, (resA, atomA), (resB, atomB)).
_CNSTRS = [
    (0, (0, 0), (1, 1)),
    (0, (1, 0), (2, 1)),
    (0, (0, 0), (1, 1)),
]

# The device program evaluates K=2 distance "slots" on local pose 0 of each
# core: slot k uses atom rows (bco[k] + 0, bco[k+1] + 1).  Each constant
# constraint must map onto one of these slots; its score contributes weight 1
# to its pose.  Verify the constant table matches this structure.
N_SLOTS = 2
for _pose, (_ra, _aa), (_rb, _ab) in _CNSTRS:
    assert _pose % PLOC == 0, "constraints must sit on local pose 0"
    assert (_aa, _ab) == (0, 1) and _rb == _ra + 1 and 0 <= _ra < N_SLOTS


def _slot_weights() -> list[np.ndarray]:
    """Per-core [N_SLOTS, PLOC] weight tables mapping distance-slot scores to
    local poses.  Derived purely from the module's constant constraint table."""
    w = [np.zeros((N_SLOTS, PLOC), np.float32) for _ in range(NCORES)]
    for pose, (ra, _aa), (_rb, _ab) in _CNSTRS:
        w[pose // PLOC][ra, pose % PLOC] += 1.0
    return w


_W_TABLES = _slot_weights()

_CACHE: dict = {}


def _build_bass():
    """Raw Bass program (no Tile): a single semaphore carries the linear
    dependency chain, so every instruction needs at most one sync-wait (the
    HW limit that Tile's auto-scheduling violates for this kernel), and the
    kernel tail is one engine barrier instead of Tile's drain butterfly.

    Semaphore ledger (DMA completions +16, compute +1).  The SWDGE gathers
    need their own semaphore (a SW-DMA sem must start from 0).  Each gather
    gets its own index tile read with a zero AP offset: the HW descriptor
    lowering drops a partition offset on the indirect-offset AP (the
    interpreter honors it, the device does not).

    Latency tricks (from NTFF traces of earlier versions):
      * no index DMA and no SWDGE indirect gathers at all: each HWDGE engine
        reg_loads its block offsets straight from DRAM and issues dynamic-
        slice (register-offset) row DMAs -- SP fetches the two A endpoints,
        ACT the two B endpoints through a statically +1-shifted coords[1:]
        view.  This removes the index-DMA completion (~1us), the SWDGE ring
        spin-up (~1.2us), the serial ~1us-per-gather SWDGE transfers, and
        one full DMA-completion hop,
      * a dummy Sqrt on the scalar engine triggers the ~1.3us PWP activation
        table load while the gather DMAs are in flight; the weights DMA gets
        its own semaphore (the PE dependency rejoins via a DVE wait before
        the score multiply),
      * all arithmetic except sqrt runs on the DVE (~150ns/op vs ~570ns/op
        on ACT), including the PSUM->SBUF copy of the matmul result,
      * no final wait on the output DMA completion: the SP drain at block
        exit flushes the queue, so the ~1us completion latency overlaps the
        teardown instead of extending the critical path.

      sem:   ga0/ga1/gb0/gb1 row dmas +64 -> 64   sub -> 65   mul -> 66
             reduce(d2) -> 67   sqrt -> 68   add(-IDEAL) -> 69
             mul(score) -> 70   matmul -> 71   psum copy -> 72
             out dma +16 -> 88 (no waiter; drained at exit)
      sem_w: wt dma +16 -> 16
    """
    import concourse.bass as bass
    import concourse.mybir as mybir

    nc = bass.Bass()
    f32 = mybir.dt.float32

    coords = nc.dram_tensor(
        "coords", [PLOC * NATOMS, 3], f32, kind="ExternalInput"
    )
    bco = nc.dram_tensor(
        "bco", [PLOC * NBLOCKS], mybir.dt.int32, kind="ExternalInput"
    )
    w = nc.dram_tensor("w", [N_SLOTS, PLOC], f32, kind="ExternalInput")
    out_t = nc.dram_tensor("out", [1, PLOC], f32, kind="ExternalOutput")

    coords_b = coords[1:]  # +1-row shifted view for the B (atom 1) endpoints

    with (
        nc.sbuf_tensor([N_SLOTS, PLOC], f32) as wt,
        nc.sbuf_tensor([N_SLOTS, 3], f32) as ga,
        nc.sbuf_tensor([N_SLOTS, 3], f32) as gb,
        nc.sbuf_tensor([N_SLOTS, 3], f32) as diff,
        nc.sbuf_tensor([N_SLOTS, 3], f32) as diffsq,
        nc.sbuf_tensor([N_SLOTS, 1], f32) as d2,
        nc.sbuf_tensor([N_SLOTS, 1], f32) as dist,
        nc.sbuf_tensor([N_SLOTS, 1], f32) as dm4,
        nc.sbuf_tensor([N_SLOTS, 1], f32) as score,
        nc.sbuf_tensor([N_SLOTS, 1], f32) as scratch,
        nc.sbuf_tensor([1, PLOC], f32) as osb,
        nc.psum_tensor([1, PLOC], f32) as op,
        nc.semaphore("s") as sem,
        nc.semaphore("sw") as sem_w,
        nc.sync.register() as ra0,
        nc.sync.register() as ra1,
        nc.scalar.register() as rb0,
        nc.scalar.register() as rb1,
        nc.Block(no_gpsimd_drain=True) as block,
    ):

        @block.sync
        def _(sync):
            # A endpoints: rows bco[0], bco[1] (atom 0)
            sync.reg_load(ra0, bco[0:1, None])
            sync.reg_load(ra1, bco[1:2, None])
            va0 = sync.snap(ra0, min_val=0, max_val=NATOMS - 1)
            sync.dma_start(out=ga[0:1, :], in_=coords[bass.ds(va0, 1), :]).then_inc(
                sem, 16
            )
            va1 = sync.snap(ra1, min_val=0, max_val=NATOMS - 1)
            sync.dma_start(out=ga[1:2, :], in_=coords[bass.ds(va1, 1), :]).then_inc(
                sem, 16
            )
            sync.wait_ge(sem, 72)
            sync.dma_start(out=out_t[:, :], in_=osb[:, :]).then_inc(sem, 16)

        @block.scalar
        def _(scalar):
            # B endpoints: rows bco[1]+1, bco[2]+1 (atom 1, via coords[1:])
            scalar.reg_load(rb0, bco[1:2, None])
            scalar.reg_load(rb1, bco[2:3, None])
            vb0 = scalar.snap(rb0, min_val=0, max_val=NATOMS - 1)
            scalar.dma_start(
                out=gb[0:1, :], in_=coords_b[bass.ds(vb0, 1), :]
            ).then_inc(sem, 16)
            vb1 = scalar.snap(rb1, min_val=0, max_val=NATOMS - 1)
            scalar.dma_start(
                out=gb[1:2, :], in_=coords_b[bass.ds(vb1, 1), :]
            ).then_inc(sem, 16)
            # warm the PWP activation table while the row DMAs are in flight
            # (scale=0.0 so the dummy never reads the uninitialized scratch)
            scalar.activation(
                out=scratch[:, :],
                in_=scratch[:, :],
                func=mybir.ActivationFunctionType.Sqrt,
                scale=0.0,
            )
            scalar.dma_start(out=wt[:, :], in_=w[:, :]).then_inc(sem_w, 16)
            scalar.wait_ge(sem, 67)
            scalar.sqrt(out=dist[:, :], in_=d2[:, :]).then_inc(sem, 1)

        @block.vector
        def _(vector):
            # d2_k = |a_k - b_k|^2
            vector.wait_ge(sem, 64)
            vector.tensor_sub(out=diff[:, :], in0=ga[:, :], in1=gb[:, :]).then_inc(
                sem, 1
            )
            vector.wait_ge(sem, 65)
            vector.tensor_mul(
                out=diffsq[:, :], in0=diff[:, :], in1=diff[:, :]
            ).then_inc(sem, 1)
            vector.wait_ge(sem, 66)
            vector.reduce_sum(
                out=d2[:, :], in_=diffsq[:, :], axis=mybir.AxisListType.X
            ).then_inc(sem, 1)
            # score_k = (dist_k - IDEAL)^2
            vector.wait_ge(sem, 68)
            vector.tensor_scalar_add(
                out=dm4[:, :], in0=dist[:, :], scalar1=-IDEAL
            ).then_inc(sem, 1)
            vector.wait_ge(sem_w, 16)
            vector.wait_ge(sem, 69)
            vector.tensor_mul(out=score[:, :], in0=dm4[:, :], in1=dm4[:, :]).then_inc(
                sem, 1
            )
            vector.wait_ge(sem, 71)
            vector.tensor_copy(out=osb[:, :], in_=op[:, :]).then_inc(sem, 1)

        @block.tensor
        def _(tensor):
            # out[p] = sum_k score[k] * w[k, p]  (wt covered transitively: the
            # score multiply is preceded by the sem_w wait on the DVE)
            tensor.wait_ge(sem, 70)
            tensor.matmul(
                out=op[:, :], lhsT=score[:, :], rhs=wt[:, :], start=True, stop=True
            ).then_inc(sem, 1)

    return nc


def _get_nc():
    if "nc" not in _CACHE:
        _CACHE["nc"] = _build_bass()
    return _CACHE["nc"]


def _in_maps(coords: np.ndarray, block_coord_offset: np.ndarray):
    maps = []
    for c in range(NCORES):
        maps.append(
            {
                "coords": np.ascontiguousarray(
                    coords[c * PLOC : (c + 1) * PLOC].reshape(PLOC * NATOMS, 3),
                    dtype=np.float32,
                ),
                "bco": np.ascontiguousarray(
                    block_coord_offset[c * PLOC : (c + 1) * PLOC].reshape(-1),
                    dtype=np.int32,
                ),
                "w": _W_TABLES[c],
            }
        )
    return maps


def run(coords: np.ndarray, block_coord_offset: np.ndarray, **run_kwargs):
    """Run on the 8 NeuronCores; returns (output [1, NPOSES], BassKernelResults)."""
    from concourse.bass_utils import run_bass_kernel_spmd

    nc = _get_nc()
    res = run_bass_kernel_spmd(
        nc,
        _in_maps(np.asarray(coords), np.asarray(block_coord_offset)),
        core_ids=list(range(NCORES)),
        **run_kwargs,
    )
    full = np.zeros((1, NPOSES), np.float32)
    for c in range(NCORES):
        full[0, c * PLOC : (c + 1) * PLOC] = res.results[c]["out"][0]
    return full, res


def kernel(coords: np.ndarray, block_coord_offset: np.ndarray) -> np.ndarray:
    full, _ = run(coords, block_coord_offset)
    return full


# revision 35
# speedup vs baseline: 1.0309x; 1.0309x over previous
"""Trainium2 Bass kernel for nn_ConstraintWholePoseScoringModule.

The module scores 3 hardcoded harmonic distance constraints (all on pose 0),
scatter-adds the scores into a [nposes, nblocks, nblocks] block-score matrix,
then sums that matrix per pose -> output [1, nposes].  The scatter + full sum
is algebraically a weighted sum of the constraint scores per pose, so the
kernel never materialises the block-score matrix.

Sharding (per the data-parallel hint): pose dimension split across 8 cores,
2 poses per core, no cross-core communication.  Every core runs the same
program on its shard:

  1. DMA the first 3 block offsets of its local pose 0 (the only pose that
     can host constraints, per the module's constant table).
  2. Two indirect-DMA gathers fetch the constraint endpoint atoms straight
     from DRAM coords (row = block_coord_offset[r] + atom; the +1 atom
     offset of the B endpoints is folded into the DMA element_offset).
  3. diff -> squared-norm (fused square+accumulate) -> sqrt -> (d-4)^2.
  4. A tiny matmul contracts the 2 distance-slot scores against a per-core
     [slot, local_pose] weight table (zeros on cores with no constraints),
     giving that core's [1, 2] per-pose output.

Host side only slices inputs per core and concatenates the [2]-vectors.
"""

import sys

sys.path.insert(0, "/opt/trn_rl_repo")

import numpy as np

NCORES = 8
NPOSES = 16
NBLOCKS = 1024
ATOMS_PER_BLOCK = 16
NATOMS = NBLOCKS * ATOMS_PER_BLOCK  # 16384
PLOC = NPOSES // NCORES  # poses per core = 2
IDEAL = 4.0

# Constant constraint table of the torch module: (pose, (resA, atomA), (resB, atomB)).
_CNSTRS = [
    (0, (0, 0), (1, 1)),
    (0, (1, 0), (2, 1)),
    (0, (0, 0), (1, 1)),
]

# The device program evaluates K=2 distance "slots" on local pose 0 of each
# core: slot k uses atom rows (bco[k] + 0, bco[k+1] + 1).  Each constant
# constraint must map onto one of these slots; its score contributes weight 1
# to its pose.  Verify the constant table matches this structure.
N_SLOTS = 2
for _pose, (_ra, _aa), (_rb, _ab) in _CNSTRS:
    assert _pose % PLOC == 0, "constraints must sit on local pose 0"
    assert (_aa, _ab) == (0, 1) and _rb == _ra + 1 and 0 <= _ra < N_SLOTS


def _slot_weights() -> list[np.ndarray]:
    """Per-core [N_SLOTS, PLOC] weight tables mapping distance-slot scores to
    local poses.  Derived purely from the module's constant constraint table."""
    w = [np.zeros((N_SLOTS, PLOC), np.float32) for _ in range(NCORES)]
    for pose, (ra, _aa), (_rb, _ab) in _CNSTRS:
        w[pose // PLOC][ra, pose % PLOC] += 1.0
    return w


_W_TABLES = _slot_weights()

_CACHE: dict = {}


def _build_bass():
    """Raw Bass program (no Tile): a single semaphore carries the linear
    dependency chain, so every instruction needs at most one sync-wait (the
    HW limit that Tile's auto-scheduling violates for this kernel), and the
    kernel tail is one engine barrier instead of Tile's drain butterfly.

    Latency tricks (from NTFF traces of earlier versions):
      * the index tile holds (bco[0],bco[1]),(bco[1],bco[2]) via one
        overlapped-AP HWDGE transfer on SP's queue (the single SWDGE ring
        does NOT order a gather's index fetch after a prior descriptor's
        write -- measured wrong results on HW -- so the gathers must wait
        on the index DMA's completion semaphore),
      * two SWDGE indirect gathers fetch the endpoint atoms (the HW DGE
        reads out.free_size consecutive elements per PARTITION index, so A
        and B endpoints cannot come from one gather); B's +1-row atom
        offset rides the DMA element_offset,
      * a dummy Sqrt on the scalar engine triggers the ~1.3us PWP activation
        table load early; the weights DMA gets its own semaphore (the PE
        dependency rejoins via a DVE wait before the score multiply),
      * |diff|^2 is one fused DVE scalar_tensor_tensor (bypass/mult with
        accum_out); the rest of the arithmetic except sqrt also runs on the
        DVE (~150ns/op vs ~570ns/op on ACT),
      * the [1, 2] result goes out via DMA with a final completion wait
        (engine reg_save stores racing NEFF teardown hard-crash the device).

      sem:   idx dma +16 -> 16   sub -> 17   stt(d2) -> 18   sqrt -> 19
             add(-IDEAL) -> 20   mul(score) -> 21   matmul -> 22
             psum copy -> 23   out dma +16 -> 39
      sem_z: zb memset -> 1
      sem_w: wt dma +16 -> 16
      sem_g: gathers +16 each -> 32 (SW-DMA semaphore)
    """
    import concourse.bass as bass
    import concourse.mybir as mybir

    # Skip the ~1.2us all-engine barrier Bass.__init__ emits after its
    # const-AP memsets: this kernel never reads the const tables (the only
    # float-bias activations take an explicit zero-bias AP that gpsimd
    # memsets under the kernel's own semaphore chain).
    _orig_aeb = bass.Bass.all_engine_barrier
    bass.Bass.all_engine_barrier = lambda self, **kw: None
    try:
        nc = bass.Bass()
    finally:
        bass.Bass.all_engine_barrier = _orig_aeb
    f32 = mybir.dt.float32

    coords = nc.dram_tensor(
        "coords", [PLOC * NATOMS, 3], f32, kind="ExternalInput"
    )
    bco = nc.dram_tensor(
        "bco", [PLOC * NBLOCKS], mybir.dt.int32, kind="ExternalInput"
    )
    w = nc.dram_tensor("w", [N_SLOTS, PLOC], f32, kind="ExternalInput")
    out_t = nc.dram_tensor("out", [1, PLOC], f32, kind="ExternalOutput")

    from contextlib import ExitStack

    with ExitStack() as ctx:
        e = ctx.enter_context
        wt = e(nc.sbuf_tensor("wt", [N_SLOTS, PLOC], f32))
        idx = e(nc.sbuf_tensor("idx", [N_SLOTS, 2], mybir.dt.int32))
        ga = e(nc.sbuf_tensor("ga", [N_SLOTS, 3], f32))
        gb = e(nc.sbuf_tensor("gb", [N_SLOTS, 3], f32))
        diff = e(nc.sbuf_tensor("diff", [N_SLOTS, 3], f32))
        diffsq = e(nc.sbuf_tensor("diffsq", [N_SLOTS, 3], f32))
        d2 = e(nc.sbuf_tensor("d2", [N_SLOTS, 1], f32))
        dist = e(nc.sbuf_tensor("dist", [N_SLOTS, 1], f32))
        dm4 = e(nc.sbuf_tensor("dm4", [N_SLOTS, 1], f32))
        score = e(nc.sbuf_tensor("score", [N_SLOTS, 1], f32))
        scratch = e(nc.sbuf_tensor("scratch", [N_SLOTS, 1], f32))
        zb = e(nc.sbuf_tensor("zb", [N_SLOTS, 1], f32))
        osb = e(nc.sbuf_tensor("osb", [1, PLOC], f32))
        op = e(nc.psum_tensor("op", [1, PLOC], f32))
        sem = e(nc.semaphore("s"))
        sem_g = e(nc.semaphore("sg"))
        sem_w = e(nc.semaphore("sw"))
        sem_z = e(nc.semaphore("sz"))
        block = e(nc.Block(no_gpsimd_drain=True))

        @block.sync
        def _(sync):
            # idx[k] = (bco[k], bco[k+1]): one overlapped-AP transfer
            sync.dma_start(
                out=idx[:, :], in_=bass.AP(bco, 0, [[1, N_SLOTS], [1, 2]])
            ).then_inc(sem, 16)
            sync.dma_start(out=wt[:, :], in_=w[:, :]).then_inc(sem_w, 16)
            sync.wait_ge(sem, 23)
            sync.dma_start(out=out_t[:, :], in_=osb[:, :]).then_inc(sem, 16)
            sync.wait_ge(sem, 39)

        @block.gpsimd
        def _(gpsimd):
            gpsimd.memset(zb[:, :], 0.0).then_inc(sem_z, 1)
            gpsimd.wait_ge(sem, 16)
            # Gather endpoint atoms straight from DRAM: row = bco[r] + atom.
            # A endpoints: blocks 0..K-1, atom 0.  B endpoints: blocks 1..K,
            # atom 1 (+1 row == +3 elements via element_offset).
            gpsimd.indirect_dma_start(
                out=ga[:, :],
                out_offset=None,
                in_=coords[:, :],
                in_offset=bass.IndirectOffsetOnAxis(ap=idx[:, 0:1], axis=0),
            ).then_inc(sem_g, 16)
            gpsimd.indirect_dma_start(
                out=gb[:, :],
                out_offset=None,
                in_=coords[:, :],
                in_offset=bass.IndirectOffsetOnAxis(ap=idx[:, 1:2], axis=0),
                element_offset=3,
            ).then_inc(sem_g, 16)

        @block.vector
        def _(vector):
            # d2_k = |A_k - B_k|^2  (fused square + accumulate)
            vector.wait_ge(sem_g, 32)
            vector.tensor_sub(out=diff[:, :], in0=ga[:, :], in1=gb[:, :]).then_inc(
                sem, 1
            )
            vector.wait_ge(sem, 17)
            vector.scalar_tensor_tensor(
                out=diffsq[:, :],
                in0=diff[:, :],
                scalar=0.0,
                in1=diff[:, :],
                op0=mybir.AluOpType.bypass,
                op1=mybir.AluOpType.mult,
                accum_out=d2[:, :],
            ).then_inc(sem, 1)
            # score_k = (dist_k - IDEAL)^2
            vector.wait_ge(sem, 19)
            vector.tensor_scalar_add(
                out=dm4[:, :], in0=dist[:, :], scalar1=-IDEAL
            ).then_inc(sem, 1)
            vector.wait_ge(sem_w, 16)
            vector.wait_ge(sem, 20)
            vector.tensor_mul(out=score[:, :], in0=dm4[:, :], in1=dm4[:, :]).then_inc(
                sem, 1
            )
            vector.wait_ge(sem, 22)
            vector.tensor_copy(out=osb[:, :], in_=op[:, :]).then_inc(sem, 1)


        @block.scalar
        def _(scalar):
            # warm the PWP activation table (scale=0.0 so the dummy never
            # reads the uninitialized scratch)
            scalar.wait_ge(sem_z, 1)
            scalar.activation(
                out=scratch[:, :],
                in_=scratch[:, :],
                func=mybir.ActivationFunctionType.Sqrt,
                scale=0.0,
                bias=zb[:, 0:1],
            )
            scalar.wait_ge(sem, 18)
            scalar.activation(
                out=dist[:, :],
                in_=d2[:, :],
                func=mybir.ActivationFunctionType.Sqrt,
                bias=zb[:, 0:1],
            ).then_inc(sem, 1)

        @block.tensor
        def _(tensor):
            # out[p] = sum_k score[k] * w[k, p]  (wt covered transitively: the
            # score multiply is preceded by the sem_w wait on the DVE)
            tensor.wait_ge(sem, 21)
            tensor.matmul(
                out=op[:, :], lhsT=score[:, :], rhs=wt[:, :], start=True, stop=True
            ).then_inc(sem, 1)

    return nc


def _get_nc():
    if "nc" not in _CACHE:
        _CACHE["nc"] = _build_bass()
    return _CACHE["nc"]


def _in_maps(coords: np.ndarray, block_coord_offset: np.ndarray):
    maps = []
    for c in range(NCORES):
        maps.append(
            {
                "coords": np.ascontiguousarray(
                    coords[c * PLOC : (c + 1) * PLOC].reshape(PLOC * NATOMS, 3),
                    dtype=np.float32,
                ),
                "bco": np.ascontiguousarray(
                    block_coord_offset[c * PLOC : (c + 1) * PLOC].reshape(-1),
                    dtype=np.int32,
                ),
                "w": _W_TABLES[c],
            }
        )
    return maps


def run(coords: np.ndarray, block_coord_offset: np.ndarray, **run_kwargs):
    """Run on the 8 NeuronCores; returns (output [1, NPOSES], BassKernelResults)."""
    from concourse.bass_utils import run_bass_kernel_spmd

    nc = _get_nc()
    res = run_bass_kernel_spmd(
        nc,
        _in_maps(np.asarray(coords), np.asarray(block_coord_offset)),
        core_ids=list(range(NCORES)),
        **run_kwargs,
    )
    full = np.zeros((1, NPOSES), np.float32)
    for c in range(NCORES):
        full[0, c * PLOC : (c + 1) * PLOC] = res.results[c]["out"][0]
    return full, res


def kernel(coords: np.ndarray, block_coord_offset: np.ndarray) -> np.ndarray:
    full, _ = run(coords, block_coord_offset)
    return full

